# revision 2
# baseline (speedup 1.0000x reference)
"""MoE layer (dense routing, 8 experts) on 8 TRN2 NeuronCores — v2.

Same expert-parallel decomposition as the baseline (core e owns expert e,
x + gating replicated, host sums the 8 partial outputs), with the per-tile
loop restructured to keep the PE stream dense:

  - The gate chain for batch tile t+1 (logit matmuls -> exp -> den/num
    matmuls -> reciprocal/mul -> gate transposes) is issued DURING tile t's
    GEMM work, interleaved between GEMM1 chains, so every cross-engine hop
    has tens of microseconds of slack instead of sitting on the PE critical
    path.
  - The gate is applied per-partition in the GEMM2 epilogue
    (y = (psum + b2_bcast) * g[b]) instead of being folded into h: that
    deletes the 32 per-tile VectorE h multiplies.
  - GEMM1 PSUM evacuation (relu + per-partition b1 bias) is split between
    ScalarE (activation) and VectorE (tensor_scalar add+max), with ph=3
    PSUM buffers, so a slow ACT never stalls the PE chain pipeline.
  - x tiles are prefetched two batch tiles ahead; y is stored as bf16
    (host accumulates in fp64) halving output DMA traffic.
"""

import numpy as np
import ml_dtypes

import concourse.bacc as bacc
import concourse.mybir as mybir
import concourse.tile as tile
from concourse.bass_utils import run_bass_kernel_spmd

B, D_IN, D_HID, D_OUT, E = 8192, 1024, 4096, 1024, 8
NCORES = 8
BT = 512                 # batch tile (matmul moving free dim)
P = 128
KD = D_IN // P           # 8 contraction subtiles for GEMM1
NH = D_HID // P          # 32 hidden tiles
NO = D_OUT // BT         # 2 output column tiles
MSUB = BT // P           # 4 output row subtiles per batch tile

BF16 = mybir.dt.bfloat16
F32 = mybir.dt.float32
AF = mybir.ActivationFunctionType
ALU = mybir.AluOpType

nbf16 = ml_dtypes.bfloat16


def build_nc(batch=B, passes=1):
    assert batch % BT == 0
    nb = batch // BT

    nc = bacc.Bacc(trn_type="TRN2")

    xt_d = nc.dram_tensor("xt", [D_IN, batch], BF16, kind="ExternalInput")
    w1_d = nc.dram_tensor("w1", [D_IN, D_HID], BF16, kind="ExternalInput")
    b1_d = nc.dram_tensor("b1c", [P, NH], F32, kind="ExternalInput")
    w2_d = nc.dram_tensor("w2", [D_HID, D_OUT], BF16, kind="ExternalInput")
    b2_d = nc.dram_tensor("b2r", [1, D_OUT], BF16, kind="ExternalInput")
    wg_d = nc.dram_tensor("wg", [KD, P, E], BF16, kind="ExternalInput")
    bg_d = nc.dram_tensor("bgc", [E, 1], F32, kind="ExternalInput")
    ones_d = nc.dram_tensor("ones", [P, P], BF16, kind="ExternalInput")
    sel_d = nc.dram_tensor("sel", [P, P], BF16, kind="ExternalInput")
    id_d = nc.dram_tensor("ident", [P, P], BF16, kind="ExternalInput")
    y_d = nc.dram_tensor("y", [batch, D_OUT], BF16, kind="ExternalOutput")

    with tile.TileContext(nc) as tc:
        with (
            tc.tile_pool(name="const", bufs=1) as const,
            tc.tile_pool(name="wpool", bufs=1) as wpool,
            tc.tile_pool(name="xpool", bufs=24) as xpool,
            tc.tile_pool(name="hpool", bufs=33) as hpool,
            tc.tile_pool(name="gpool", bufs=2) as gpool,
            tc.tile_pool(name="gbpool", bufs=2) as gbpool,
            tc.tile_pool(name="ypool", bufs=6) as ypool,
            tc.tile_pool(name="gppool", bufs=8) as gppool,
            tc.tile_pool(name="ph", bufs=3, space="PSUM") as ph,
            tc.tile_pool(name="py", bufs=3, space="PSUM") as py,
            tc.tile_pool(name="pg", bufs=2, space="PSUM") as pg,
        ):
            # ---- persistent tiles -------------------------------------
            wg_sb = const.tile([P, KD, E], BF16, tag="wg")
            nc.sync.dma_start(wg_sb[:], wg_d[:].rearrange("k p e -> p k e"))
            b1_sb = const.tile([P, NH], F32, tag="b1")
            nc.sync.dma_start(b1_sb[:], b1_d[:])
            bg_sb = const.tile([E, 1], F32, tag="bg")
            nc.sync.dma_start(bg_sb[:], bg_d[:])
            ones_sb = const.tile([P, P], BF16, tag="ones")
            nc.sync.dma_start(ones_sb[:], ones_d[:])
            sel_sb = const.tile([P, P], BF16, tag="sel")
            nc.sync.dma_start(sel_sb[:], sel_d[:])
            id_sb = const.tile([P, P], BF16, tag="ident")
            nc.sync.dma_start(id_sb[:], id_d[:])

            # x tiles, prefetched 2 batch tiles ahead (one alloc per kd)
            def load_x(bt):
                b0 = bt * BT
                ts = []
                for kd in range(KD):
                    t = xpool.tile([P, BT], BF16, tag="xt")
                    nc.sync.dma_start(
                        t[:], xt_d[kd * P:(kd + 1) * P, b0:b0 + BT])
                    ts.append(t)
                return ts

            tiles_seq = [i for _ in range(passes) for i in range(nb)]
            n_it = len(tiles_seq)
            x_of = {0: load_x(tiles_seq[0])}
            if n_it > 1:
                x_of[1] = load_x(tiles_seq[1])

            # w1 DMAs split into column chunks, chunk-major, so the first
            # GEMM1 n-tiles become runnable after ~2MB instead of 8MB.
            w1_sb = [wpool.tile([P, D_HID], BF16, tag=f"w1_{kd}",
                                name=f"w1_{kd}")
                     for kd in range(KD)]
            W1C = 4
            for c in range(W1C):
                cs = slice(c * (D_HID // W1C), (c + 1) * (D_HID // W1C))
                for kd in range(KD):
                    nc.sync.dma_start(w1_sb[kd][:, cs],
                                      w1_d[kd * P:(kd + 1) * P, cs])
            w2_sb = []
            for kh in range(NH):
                t = wpool.tile([P, D_OUT], BF16, tag=f"w2_{kh}")
                nc.sync.dma_start(t[:], w2_d[kh * P:(kh + 1) * P, :])
                w2_sb.append(t)
            # b2 broadcast to all partitions once, via a ones-matmul against a
            # zero-padded single-row staging tile.
            w2x = const.tile([P, D_OUT], BF16, tag="w2x")
            nc.vector.memset(w2x[:], 0.0)
            nc.sync.dma_start(w2x[0:1, :], b2_d[:])
            b2bc = const.tile([P, D_OUT], F32, tag="b2bc")
            for ot in range(NO):
                pb2 = pg.tile([P, BT], F32, tag="g", name=f"pb2_{ot}")
                nc.tensor.matmul(pb2[:], lhsT=ones_sb[:],
                                 rhs=w2x[:, ot * BT:(ot + 1) * BT],
                                 start=True, stop=True)
                nc.any.tensor_copy(out=b2bc[:, ot * BT:(ot + 1) * BT],
                                   in_=pb2[:])

            # exp of gate logits, zero-padded to full 128 partitions so the
            # broadcast matmuls contract over K=128
            exp_sb = const.tile([P, BT], BF16, tag="exp")
            nc.vector.memset(exp_sb[:], 0.0)

            # ---- gate chain, split in three stages so it can be
            # interleaved with GEMM1 of the previous tile -----------------
            def gate_logits(xts):
                lg = pg.tile([E, BT], F32, tag="g")
                for kd in range(KD):
                    nc.tensor.matmul(
                        lg[:], lhsT=wg_sb[:, kd, :], rhs=xts[kd][:],
                        start=(kd == 0), stop=(kd == KD - 1))
                nc.scalar.activation(exp_sb[0:E, :], lg[:], AF.Exp,
                                     bias=bg_sb[:], scale=1.0)

            def gate_reduce():
                den = pg.tile([P, BT], F32, tag="g")
                nc.tensor.matmul(den[:], lhsT=ones_sb[:], rhs=exp_sb[:],
                                 start=True, stop=True)
                num = pg.tile([P, BT], F32, tag="g")
                nc.tensor.matmul(num[:], lhsT=sel_sb[:], rhs=exp_sb[:],
                                 start=True, stop=True)
                rec = gpool.tile([P, BT], F32, tag="rec")
                nc.vector.reciprocal(rec[:], den[:])
                gbc = gbpool.tile([P, BT], BF16, tag="gbc")
                nc.vector.tensor_mul(out=gbc[:], in0=num[:], in1=rec[:])
                return gbc

            def gate_cols(gbc):
                # per-partition gate columns for the GEMM2 epilogue: transpose
                # each 128-wide slice of the (partition-replicated) gbc and
                # keep column 0
                gps = []
                for ms in range(MSUB):
                    tp = pg.tile([P, P], BF16, tag="g", name=f"tp{ms}")
                    nc.tensor.transpose(
                        tp[:], gbc[:, ms * P:(ms + 1) * P], id_sb[:])
                    gp = gppool.tile([P, 1], F32, tag="gp", name=f"gp{ms}")
                    nc.vector.tensor_copy(out=gp[:], in_=tp[:, 0:1])
                    gps.append(gp)
                return gps

            # gate for iteration 0 computed up front
            gate_logits(x_of[0])
            gbc0 = gate_reduce()
            gps_of = {0: gate_cols(gbc0)}

            # ---- main loop over batch tiles ---------------------------
            # passes>1 repeats the whole loop (same output) — used only by
            # the perf harness to measure device time as a wall-clock slope.
            for it in range(n_it):
                bt = tiles_seq[it]
                b0 = bt * BT
                xts = x_of.pop(it)
                gps = gps_of.pop(it)

                if it + 2 < n_it:
                    x_of[it + 2] = load_x(tiles_seq[it + 2])

                # stage A of next tile's gate: logit matmuls + exp
                if it + 1 < n_it:
                    gate_logits(x_of[it + 1])

                # GEMM1: hT[n, b] = relu(sum_d W1[d,n] xT[d,b] + b1[n]).
                # The remaining gate stages for tile t+1 are dropped between
                # chains so their producers (ScalarE exp, VectorE mul) have
                # microseconds of slack before the PE consumes them.
                gbc_next = None
                hs = []
                for nt in range(NH):
                    if nt == 4 and it + 1 < n_it:
                        gbc_next = gate_reduce()
                    if nt == 12 and it + 1 < n_it:
                        gps_of[it + 1] = gate_cols(gbc_next)
                    acc = ph.tile([P, BT], F32, tag="acc")
                    for kd in range(KD):
                        nc.tensor.matmul(
                            acc[:],
                            lhsT=w1_sb[kd][:, nt * P:(nt + 1) * P],
                            rhs=xts[kd][:],
                            start=(kd == 0), stop=(kd == KD - 1))
                    h = hpool.tile([P, BT], BF16, tag="h")
                    if nt % 3 == 0:
                        nc.scalar.activation(h[:], acc[:], AF.Relu,
                                             bias=b1_sb[:, nt:nt + 1],
                                             scale=1.0)
                    else:
                        nc.vector.tensor_scalar(
                            out=h[:], in0=acc[:],
                            scalar1=b1_sb[:, nt:nt + 1], scalar2=0.0,
                            op0=ALU.add, op1=ALU.max)
                    hs.append(h)

                # GEMM2: y[b, o] = sum_h hT[h, b] W2[h, o]; epilogue applies
                # bias and gate per-partition: y = (psum + b2_bcast) * g[b].
                for ms in range(MSUB):
                    accs = [py.tile([P, BT], F32, tag="acc", name=f"acc{ot}")
                            for ot in range(NO)]
                    for kh in range(NH):
                        lhsT = hs[kh][:, ms * P:(ms + 1) * P]
                        for ot in range(NO):
                            nc.tensor.matmul(
                                accs[ot][:],
                                lhsT=lhsT,
                                rhs=w2_sb[kh][:, ot * BT:(ot + 1) * BT],
                                start=(kh == 0), stop=(kh == NH - 1))
                    for ot in range(NO):
                        yt = ypool.tile([P, BT], BF16, tag="y")
                        nc.vector.tensor_tensor(
                            out=yt[:], in0=accs[ot][:],
                            in1=b2bc[:, ot * BT:(ot + 1) * BT],
                            op=ALU.add)
                        nc.vector.tensor_scalar(
                            out=yt[:], in0=yt[:], scalar1=gps[ms][:],
                            scalar2=None, op0=ALU.mult)
                        nc.sync.dma_start(
                            y_d[b0 + ms * P:b0 + (ms + 1) * P,
                                ot * BT:(ot + 1) * BT],
                            yt[:])

    nc.finalize()
    return nc


def make_in_maps(x, W1, b1, W2, b2, Wg, bg, batch=B):
    """Host-side sharding prep: transpose x once, cast matmul operands to
    bf16, reshape biases to the on-chip layouts."""
    f32 = np.float32
    xt = np.ascontiguousarray(x.astype(f32).T).astype(nbf16)      # [D_IN, B]
    wg = np.ascontiguousarray(
        Wg.astype(f32).reshape(KD, P, E)).astype(nbf16)
    bgc = np.ascontiguousarray(bg.astype(f32).reshape(E, 1))
    ones = np.ones((P, P), dtype=nbf16)
    ident = np.eye(P, dtype=nbf16)

    in_maps = []
    for e in range(NCORES):
        sel = np.zeros((P, P), dtype=nbf16)
        sel[e, :] = 1.0
        in_maps.append({
            "xt": xt,
            "w1": np.ascontiguousarray(W1[e].astype(f32)).astype(nbf16),
            "b1c": np.ascontiguousarray(
                b1[e].astype(f32).reshape(NH, P).T),
            "w2": np.ascontiguousarray(W2[e].astype(f32)).astype(nbf16),
            "b2r": np.ascontiguousarray(
                b2[e].astype(f32).reshape(1, D_OUT)).astype(nbf16),
            "wg": wg,
            "bgc": bgc,
            "ones": ones,
            "sel": sel,
            "ident": ident,
        })
    return in_maps


def kernel(x, W1, b1, W2, b2, Wg, bg):
    in_maps = make_in_maps(x, W1, b1, W2, b2, Wg, bg)
    nc = build_nc(B)
    res = run_bass_kernel_spmd(nc, in_maps, core_ids=list(range(NCORES)))
    out = res.results[0]["y"].astype(np.float64)
    for e in range(1, NCORES):
        out += res.results[e]["y"].astype(np.float64)
    return out.astype(np.float32)


# revision 3
# speedup vs baseline: 1.0061x; 1.0061x over previous
"""MoE layer (dense routing, 8 experts) on 8 TRN2 NeuronCores — v2.

Same expert-parallel decomposition as the baseline (core e owns expert e,
x + gating replicated, host sums the 8 partial outputs), with the per-tile
loop restructured to keep the PE stream dense:

  - The gate chain for batch tile t+1 (logit matmuls -> exp -> den/num
    matmuls -> reciprocal/mul -> gate transposes) is issued DURING tile t's
    GEMM work, interleaved between GEMM1 chains, so every cross-engine hop
    has tens of microseconds of slack instead of sitting on the PE critical
    path.
  - The gate is applied per-partition in the GEMM2 epilogue
    (y = (psum + b2_bcast) * g[b]) instead of being folded into h: that
    deletes the 32 per-tile VectorE h multiplies.
  - GEMM1 PSUM evacuation (relu + per-partition b1 bias) is split between
    ScalarE (activation) and VectorE (tensor_scalar add+max), with ph=3
    PSUM buffers, so a slow ACT never stalls the PE chain pipeline.
  - x tiles are prefetched two batch tiles ahead; y is stored as bf16
    (host accumulates in fp64) halving output DMA traffic.
"""

import numpy as np
import ml_dtypes

import concourse.bacc as bacc
import concourse.mybir as mybir
import concourse.tile as tile
from concourse.bass_utils import run_bass_kernel_spmd

B, D_IN, D_HID, D_OUT, E = 8192, 1024, 4096, 1024, 8
NCORES = 8
BT = 512                 # batch tile (matmul moving free dim)
P = 128
KD = D_IN // P           # 8 contraction subtiles for GEMM1
NH = D_HID // P          # 32 hidden tiles
NO = D_OUT // BT         # 2 output column tiles
MSUB = BT // P           # 4 output row subtiles per batch tile

BF16 = mybir.dt.bfloat16
F32 = mybir.dt.float32
AF = mybir.ActivationFunctionType
ALU = mybir.AluOpType

nbf16 = ml_dtypes.bfloat16


def build_nc(batch=B, passes=1):
    assert batch % BT == 0
    nb = batch // BT

    nc = bacc.Bacc(trn_type="TRN2")

    xt_d = nc.dram_tensor("xt", [D_IN, batch], BF16, kind="ExternalInput")
    w1_d = nc.dram_tensor("w1", [D_IN, D_HID], BF16, kind="ExternalInput")
    b1_d = nc.dram_tensor("b1c", [P, NH], F32, kind="ExternalInput")
    w2_d = nc.dram_tensor("w2", [D_HID, D_OUT], BF16, kind="ExternalInput")
    b2_d = nc.dram_tensor("b2c", [P, D_OUT // P], F32, kind="ExternalInput")
    wg_d = nc.dram_tensor("wg", [KD, P, E], BF16, kind="ExternalInput")
    bg_d = nc.dram_tensor("bgc", [E, 1], F32, kind="ExternalInput")
    ones_d = nc.dram_tensor("ones", [P, P], BF16, kind="ExternalInput")
    sel_d = nc.dram_tensor("sel", [P, P], BF16, kind="ExternalInput")
    id_d = nc.dram_tensor("ident", [P, P], BF16, kind="ExternalInput")
    y_d = nc.dram_tensor("yT", [D_OUT, batch], BF16, kind="ExternalOutput")

    with tile.TileContext(nc) as tc:
        with (
            tc.tile_pool(name="const", bufs=1) as const,
            tc.tile_pool(name="wpool", bufs=1) as wpool,
            tc.tile_pool(name="xpool", bufs=24) as xpool,
            tc.tile_pool(name="hpool", bufs=33) as hpool,
            tc.tile_pool(name="gpool", bufs=2) as gpool,
            tc.tile_pool(name="gbpool", bufs=2) as gbpool,
            tc.tile_pool(name="ypool", bufs=6) as ypool,
            tc.tile_pool(name="gppool", bufs=8) as gppool,
            tc.tile_pool(name="ph", bufs=3, space="PSUM") as ph,
            tc.tile_pool(name="py", bufs=3, space="PSUM") as py,
            tc.tile_pool(name="pg", bufs=2, space="PSUM") as pg,
        ):
            # ---- persistent tiles -------------------------------------
            wg_sb = const.tile([P, KD, E], BF16, tag="wg")
            nc.sync.dma_start(wg_sb[:], wg_d[:].rearrange("k p e -> p k e"))
            b1_sb = const.tile([P, NH], F32, tag="b1")
            nc.sync.dma_start(b1_sb[:], b1_d[:])
            bg_sb = const.tile([E, 1], F32, tag="bg")
            nc.sync.dma_start(bg_sb[:], bg_d[:])
            ones_sb = const.tile([P, P], BF16, tag="ones")
            nc.sync.dma_start(ones_sb[:], ones_d[:])
            sel_sb = const.tile([P, P], BF16, tag="sel")
            nc.sync.dma_start(sel_sb[:], sel_d[:])

            # x tiles, prefetched 2 batch tiles ahead (one alloc per kd)
            def load_x(bt):
                b0 = bt * BT
                ts = []
                for kd in range(KD):
                    t = xpool.tile([P, BT], BF16, tag="xt")
                    nc.sync.dma_start(
                        t[:], xt_d[kd * P:(kd + 1) * P, b0:b0 + BT])
                    ts.append(t)
                return ts

            tiles_seq = [i for _ in range(passes) for i in range(nb)]
            n_it = len(tiles_seq)
            x_of = {0: load_x(tiles_seq[0])}
            if n_it > 1:
                x_of[1] = load_x(tiles_seq[1])

            # w1 DMAs split into column chunks, chunk-major, so the first
            # GEMM1 n-tiles become runnable after ~2MB instead of 8MB.
            w1_sb = [wpool.tile([P, D_HID], BF16, tag=f"w1_{kd}",
                                name=f"w1_{kd}")
                     for kd in range(KD)]
            W1C = 4
            for c in range(W1C):
                cs = slice(c * (D_HID // W1C), (c + 1) * (D_HID // W1C))
                for kd in range(KD):
                    nc.sync.dma_start(w1_sb[kd][:, cs],
                                      w1_d[kd * P:(kd + 1) * P, cs])
            w2_sb = []
            for kh in range(NH):
                t = wpool.tile([P, D_OUT], BF16, tag=f"w2_{kh}")
                nc.sync.dma_start(t[:], w2_d[kh * P:(kh + 1) * P, :])
                w2_sb.append(t)
            # b2 reshaped host-side to per-partition columns [P, D_OUT/P]
            b2c_sb = const.tile([P, D_OUT // P], F32, tag="b2c")
            nc.sync.dma_start(b2c_sb[:], b2_d[:])

            # exp of gate logits, zero-padded to full 128 partitions so the
            # broadcast matmuls contract over K=128
            exp_sb = const.tile([P, BT], BF16, tag="exp")
            nc.vector.memset(exp_sb[:], 0.0)

            # ---- gate chain, split in three stages so it can be
            # interleaved with GEMM1 of the previous tile -----------------
            def gate_logits(xts):
                lg = pg.tile([E, BT], F32, tag="g")
                for kd in range(KD):
                    nc.tensor.matmul(
                        lg[:], lhsT=wg_sb[:, kd, :], rhs=xts[kd][:],
                        start=(kd == 0), stop=(kd == KD - 1))
                nc.scalar.activation(exp_sb[0:E, :], lg[:], AF.Exp,
                                     bias=bg_sb[:], scale=1.0)

            def gate_reduce():
                den = pg.tile([P, BT], F32, tag="g")
                nc.tensor.matmul(den[:], lhsT=ones_sb[:], rhs=exp_sb[:],
                                 start=True, stop=True)
                num = pg.tile([P, BT], F32, tag="g")
                nc.tensor.matmul(num[:], lhsT=sel_sb[:], rhs=exp_sb[:],
                                 start=True, stop=True)
                rec = gpool.tile([P, BT], F32, tag="rec")
                nc.vector.reciprocal(rec[:], den[:])
                gbc = gbpool.tile([P, BT], BF16, tag="gbc")
                nc.vector.tensor_mul(out=gbc[:], in0=num[:], in1=rec[:])
                return gbc

            # gate for iteration 0 computed up front
            gate_logits(x_of[0])
            gbc_of = {0: gate_reduce()}

            # ---- main loop over batch tiles ---------------------------
            # passes>1 repeats the whole loop (same output) — used only by
            # the perf harness to measure device time as a wall-clock slope.
            for it in range(n_it):
                bt = tiles_seq[it]
                b0 = bt * BT
                xts = x_of.pop(it)
                gbc = gbc_of.pop(it)

                if it + 2 < n_it:
                    x_of[it + 2] = load_x(tiles_seq[it + 2])

                # stage A of next tile's gate: logit matmuls + exp
                if it + 1 < n_it:
                    gate_logits(x_of[it + 1])

                # GEMM1: hT[n, b] = relu(sum_d W1[d,n] xT[d,b] + b1[n]).
                # The remaining gate stages for tile t+1 are dropped between
                # chains so their producers (ScalarE exp, VectorE mul) have
                # microseconds of slack before the PE consumes them.
                hs = []
                for nt in range(NH):
                    if nt == 4 and it + 1 < n_it:
                        gbc_of[it + 1] = gate_reduce()
                    acc = ph.tile([P, BT], F32, tag="acc")
                    for kd in range(KD):
                        nc.tensor.matmul(
                            acc[:],
                            lhsT=w1_sb[kd][:, nt * P:(nt + 1) * P],
                            rhs=xts[kd][:],
                            start=(kd == 0), stop=(kd == KD - 1))
                    h = hpool.tile([P, BT], BF16, tag="h")
                    if nt % 3 == 0:
                        nc.scalar.activation(h[:], acc[:], AF.Relu,
                                             bias=b1_sb[:, nt:nt + 1],
                                             scale=1.0)
                    else:
                        nc.vector.tensor_scalar(
                            out=h[:], in0=acc[:],
                            scalar1=b1_sb[:, nt:nt + 1], scalar2=0.0,
                            op0=ALU.add, op1=ALU.max)
                    hs.append(h)

                # GEMM2 transposed: yT[o, b] = sum_h W2[h, o]^T hT[h, b];
                # W2 128-col blocks are the stationary operand, h tiles the
                # moving one.  In this orientation b2 is a per-partition
                # scalar and the gate is the partition-replicated gbc tile,
                # so the epilogue is one scalar_tensor_tensor:
                #   yT = (psum + b2[o]) * g[b].
                for ob in range(D_OUT // P):
                    acc2 = py.tile([P, BT], F32, tag="acc")
                    for kh in range(NH):
                        nc.tensor.matmul(
                            acc2[:],
                            lhsT=w2_sb[kh][:, ob * P:(ob + 1) * P],
                            rhs=hs[kh][:],
                            start=(kh == 0), stop=(kh == NH - 1))
                    yt = ypool.tile([P, BT], BF16, tag="y")
                    nc.vector.scalar_tensor_tensor(
                        out=yt[:], in0=acc2[:],
                        scalar=b2c_sb[:, ob:ob + 1],
                        in1=gbc[:],
                        op0=ALU.add, op1=ALU.mult)
                    nc.sync.dma_start(
                        y_d[ob * P:(ob + 1) * P, b0:b0 + BT], yt[:])

    nc.finalize()
    return nc


def make_in_maps(x, W1, b1, W2, b2, Wg, bg, batch=B):
    """Host-side sharding prep: transpose x once, cast matmul operands to
    bf16, reshape biases to the on-chip layouts."""
    f32 = np.float32
    xt = np.ascontiguousarray(x.astype(f32).T).astype(nbf16)      # [D_IN, B]
    wg = np.ascontiguousarray(
        Wg.astype(f32).reshape(KD, P, E)).astype(nbf16)
    bgc = np.ascontiguousarray(bg.astype(f32).reshape(E, 1))
    ones = np.ones((P, P), dtype=nbf16)
    ident = np.eye(P, dtype=nbf16)

    in_maps = []
    for e in range(NCORES):
        sel = np.zeros((P, P), dtype=nbf16)
        sel[e, :] = 1.0
        in_maps.append({
            "xt": xt,
            "w1": np.ascontiguousarray(W1[e].astype(f32)).astype(nbf16),
            "b1c": np.ascontiguousarray(
                b1[e].astype(f32).reshape(NH, P).T),
            "w2": np.ascontiguousarray(W2[e].astype(f32)).astype(nbf16),
            "b2c": np.ascontiguousarray(
                b2[e].astype(f32).reshape(D_OUT // P, P).T),
            "wg": wg,
            "bgc": bgc,
            "ones": ones,
            "sel": sel,
            "ident": ident,
        })
    return in_maps


def kernel(x, W1, b1, W2, b2, Wg, bg):
    in_maps = make_in_maps(x, W1, b1, W2, b2, Wg, bg)
    nc = build_nc(B)
    res = run_bass_kernel_spmd(nc, in_maps, core_ids=list(range(NCORES)))
    out = res.results[0]["yT"].astype(np.float64)
    for e in range(1, NCORES):
        out += res.results[e]["yT"].astype(np.float64)
    return np.ascontiguousarray(out.T).astype(np.float32)
